# revision 34
# baseline (speedup 1.0000x reference)
"""NodeMPNN (message passing + GRU + LayerNorm) on 8 Trainium2 NeuronCores.

Strategy (dst-sharded graph parallel, transfer-minimized):
  - Nodes/edges sharded by destination node across 8 cores (6250 dst/core).
  - Host ships each core ONLY its own bf16 node shard; the full node table is
    rebuilt in every core's HBM with an on-device AllGather over NeuronLink
    (the "halo exchange"), so host->device traffic is 1/8th of replicating.
  - ALL per-core inputs are packed into a single u8 blob parameter (the axon
    tunnel charges ~25ms fixed latency per jit argument; one blob instead of
    ~17 arrays). Sections are sliced on device via bitcast APs.
  - Source-feature gathers are local indirect-DMA reads of the gathered table.
  - Linearity trick: segment_sum(nodes[src] @ W^T) = segment_sum(nodes[src]) @ W^T,
    so we gather raw node rows and apply W_msg once per 512-dst block.
  - Segment sum via PE: edges sorted by dst, padded per 128-dst window;
    one-hot selection matrices built on DVE (iota is_equal against host-provided
    dst offsets); PSUM accumulates G^T @ S = messages^T per window. Pad slots
    gather row 0 with dst offset 255 (matches no one-hot column).
  - int16 gather indices: table split at row 25000 into lo/hi views (two
    streams) so indices fit int16. Index tables are shipped once ([16, n*8])
    and replicated across the 8 gpsimd channels on device.
  - GRU gates computed in transposed (feature-major) layout: gate = W_ih@msg^T +
    W_hh@nodes^T accumulated in PSUM; mean-node term folded into per-feature gate
    biases (partial sums AllReduced across cores).
  - LayerNorm row-major after PE transposes, bn_stats/bn_aggr + ACT apply.
  - Output quantized on device to u8 with a per-row f32 scale (132 B/row vs
    512 B f32): halves-again the result download AND the donated-zeros upload
    that PJRT ships for output buffers. Dequantized to f32 on host; adds
    ~0.6% rms error (total ~7e-3 « 2e-2 gate).
"""

import sys

sys.path.insert(0, "/opt/trn_rl_repo")

from contextlib import ExitStack

import numpy as np
import ml_dtypes

import concourse.bass as bass
import concourse.bacc as bacc
import concourse.tile as tile
from concourse import mybir
from concourse.bass_utils import run_bass_kernel_spmd

BF16 = ml_dtypes.bfloat16
P = 128
N_CORES = 8
WIN = 128          # dst window (one-hot width)
SB = 512           # dst super-block (PSUM free dim)
PAD_OFF = 255.0    # dst offset for pad slots: never matches iota 0..127
OUT_COLS = P + 4   # per-row output: 128 u8 quants + packed f32 inv-scale
Q_BIAS = 128.0     # u8 offset; HW ACT f32->u8 convert rounds to nearest


def _unpack_out(arr):
    """[rows, 132] u8 -> [rows, 128] f32: (q - 128) * row_inv_scale."""
    q = arr[:, :P].astype(np.float32) - 128.0
    sc = np.ascontiguousarray(arr[:, P:P + 4]).view(np.float32)
    return q * sc

_NPDT = {"bf16": BF16, "f32": np.float32, "i16": np.int16, "u8": np.uint8,
         "i8": np.int8}


def _layout(meta):
    """Section layout of the packed input blob: name -> (offset, rows, cols, dtype)."""
    H, shard_pad = meta["H"], meta["shard_pad"]
    sh32 = meta["shard32"]
    ntl, nth = meta["n_tiles_lo"], meta["n_tiles_hi"]
    secs = [
        ("shard_q", sh32, H, "i8"),
        ("shard_sc", sh32, 1, "f32"),
        ("idx_lo", 16, ntl * 8, "i16"),
        ("idx_hi", 16, nth * 8, "i16"),
        ("dst_lo", P, ntl, "u8"),
        ("dst_hi", P, nth, "u8"),
        ("ident", P, P, "bf16"),
        ("wmsgT", H, H, "bf16"),
        ("wihT", H, 3 * H, "bf16"),
        ("whhT", H, 3 * H, "bf16"),
        ("iota", 1, P, "u8"),
        ("gamma_t", 1, H, "f32"),
        ("beta_t", 1, H, "f32"),
        ("bih_t", H, 3, "f32"),
        ("bhh_t", H, 3, "f32"),
    ]
    if meta["has_bias"]:
        secs += [("deg", 1, shard_pad, "bf16"), ("bmsg_row", 1, H, "bf16")]
    out, off = {}, 0
    _ESZ = {"f32": 4, "bf16": 2, "i16": 2, "u8": 1, "i8": 1}
    for name, r, c, dt_ in secs:
        nbytes = r * c * _ESZ[dt_]
        out[name] = (off, r, c, dt_)
        off += -(-nbytes // 256) * 256
    return out, off


def _host_prep(nodes, W_msg, b_msg, w_ih, w_hh, b_ih, b_hh, ln_gamma, ln_beta,
               edge_src, edge_dst):
    """Sort/pad edges, build per-core SPMD input blobs and the tile schedule."""
    N, H = nodes.shape
    assert H == P
    assert N % N_CORES == 0
    shard = N // N_CORES                  # dst nodes per core
    shard_pad = -(-shard // SB) * SB      # padded to super-block multiple
    shard32 = -(-shard // 32) * 32        # upload pad (transpose-DMA xbar tile)
    nsb = shard_pad // SB                 # super-blocks per core
    nw = -(-shard // WIN)                 # real dst windows per core

    half = (N + 1) // 2                   # split tables: int16 gather indices
    assert half < 32768 and N - half < 32768

    has_bias = bool(np.any(np.asarray(b_msg) != 0.0))

    # --- group edges by (core, window, stream) ---
    d_s = np.asarray(edge_dst).astype(np.int64)
    s_s = np.asarray(edge_src).astype(np.int64)
    stream = (s_s >= half).astype(np.int64)
    loc = np.where(stream == 0, s_s, s_s - half)

    core = d_s // shard
    within = d_s - core * shard
    w_of = within // WIN
    off_of = within % WIN

    key = (core * nw + w_of) * 2 + stream
    order = np.argsort(key, kind="stable")
    key, loc, off_of, core = key[order], loc[order], off_of[order], core[order]
    w_s = w_of[order]
    st_s = stream[order]

    counts = np.bincount(key, minlength=N_CORES * nw * 2).reshape(N_CORES, nw, 2)
    tw = (counts.max(axis=0) + P - 1) // P           # [nw, 2] tiles per (window, stream)
    n_tiles_s = [int(tw[:, s].sum()) for s in (0, 1)]
    assert n_tiles_s[0] > 0 and n_tiles_s[1] > 0
    wstart_s = []
    for s in (0, 1):
        ws = np.zeros(nw + 1, np.int64)
        ws[1:] = np.cumsum(tw[:, s] * P)
        wstart_s.append(ws)

    starts_flat = np.zeros(N_CORES * nw * 2 + 1, np.int64)
    starts_flat[1:] = np.cumsum(counts.reshape(-1))
    rank = np.arange(d_s.shape[0], dtype=np.int64) - starts_flat[key]
    slot = np.where(st_s == 0, wstart_s[0][w_s], wstart_s[1][w_s]) + rank

    src_arrs, off_arrs = [], []
    for s in (0, 1):
        total = n_tiles_s[s] * P
        sa = np.zeros((N_CORES, total), np.int16)        # pad: gather row 0
        oa = np.full((N_CORES, total), PAD_OFF, np.float32)
        m = st_s == s
        sa[core[m], slot[m]] = loc[m]
        oa[core[m], slot[m]] = off_of[m]
        src_arrs.append(sa)
        off_arrs.append(oa)

    meta = dict(N=N, H=H, half=half, shard=shard, shard_pad=shard_pad,
                shard32=shard32, nsb=nsb,
                nw=nw, n_tiles_lo=n_tiles_s[0], n_tiles_hi=n_tiles_s[1],
                has_bias=has_bias,
                tw=[[int(tw[w, 0]), int(tw[w, 1])] for w in range(nw)],
                wstart_lo=[int(x) for x in wstart_s[0]],
                wstart_hi=[int(x) for x in wstart_s[1]])
    layout, total_bytes = _layout(meta)
    meta["total_bytes"] = total_bytes

    # --- shared (replicated) sections ---
    nodes_f32 = np.asarray(nodes, np.float32)
    shared = {
        "iota": np.arange(P, dtype=np.uint8).reshape(1, P),
        "ident": np.eye(P, dtype=np.float32).astype(BF16),
        "gamma_t": np.asarray(ln_gamma, np.float32).reshape(1, H),
        "beta_t": np.asarray(ln_beta, np.float32).reshape(1, H),
        "wmsgT": np.asarray(W_msg, np.float32).T.astype(BF16),
        "wihT": np.asarray(w_ih, np.float32).T.astype(BF16),
        "whhT": np.asarray(w_hh, np.float32).T.astype(BF16),
        "bih_t": np.asarray(b_ih, np.float32).reshape(3, H).T.astype(np.float32),
        "bhh_t": np.asarray(b_hh, np.float32).reshape(3, H).T.astype(np.float32),
    }
    if has_bias:
        deg_all = np.bincount(d_s, minlength=N).astype(np.float32)
        shared["bmsg_row"] = np.asarray(b_msg, np.float32).reshape(1, H).astype(BF16)

    in_maps = []
    for c in range(N_CORES):
        blob = np.zeros(total_bytes, np.uint8)

        def put(name, arr):
            off, r, cc, dt_ = layout[name]
            a = np.ascontiguousarray(arr, dtype=_NPDT[dt_])
            assert a.shape == (r, cc), (name, a.shape, (r, cc))
            blob[off:off + a.nbytes] = a.view(np.uint8).reshape(-1)

        rows = nodes_f32[c * shard:(c + 1) * shard]
        sc = np.maximum(np.abs(rows).max(axis=1), 1e-30) / 127.0
        q = np.zeros((shard32, H), np.int8)
        q[:shard] = np.clip(np.rint(rows / sc[:, None]), -127, 127)
        scp = np.zeros((shard32, 1), np.float32)
        scp[:shard, 0] = sc
        put("shard_q", q)
        put("shard_sc", scp)
        for s, nm in ((0, "lo"), (1, "hi")):
            flat = src_arrs[s][c]
            # wrapped int16 layout: index i at [i % 16, i // 16]
            put(f"idx_{nm}", flat.reshape(-1, 16).T)
            put(f"dst_{nm}", off_arrs[s][c].reshape(n_tiles_s[s], P).T)
        for k, v in shared.items():
            put(k, v)
        if has_bias:
            dg = np.zeros((1, shard_pad), np.float32)
            dg[0, :shard] = deg_all[c * shard:(c + 1) * shard]
            put("deg", dg)
        in_maps.append({"blob": blob})

    return in_maps, meta


def _build_program(meta):
    N, H, half = meta["N"], meta["H"], meta["half"]
    shard, shard_pad, nsb, nw = meta["shard"], meta["shard_pad"], meta["nsb"], meta["nw"]
    sh32 = meta["shard32"]
    tw = meta["tw"]
    has_bias = meta["has_bias"]
    n_tiles_s = (meta["n_tiles_lo"], meta["n_tiles_hi"])
    wstart_s = (meta["wstart_lo"], meta["wstart_hi"])
    WPSB = SB // WIN  # windows per super-block (4)
    layout, total_bytes = _layout(meta)

    nc = bacc.Bacc("TRN2", target_bir_lowering=False, debug=False,
                   num_devices=N_CORES)
    f32, bf16, i16 = mybir.dt.float32, mybir.dt.bfloat16, mybir.dt.int16
    u8, i8 = mybir.dt.uint8, mybir.dt.int8
    _BDT = {"bf16": bf16, "f32": f32, "i16": i16, "u8": u8, "i8": i8}

    blob_d = nc.declare_dram_parameter("blob", [total_bytes], u8, isOutput=False)
    out_d = nc.declare_dram_parameter("out_shard", [shard, OUT_COLS], u8, isOutput=True)

    _ESZ = {"f32": 4, "bf16": 2, "i16": 2, "u8": 1, "i8": 1}

    def bap(name, rows=None):
        off, r, c, dt_ = layout[name]
        r = rows if rows is not None else r
        return (blob_d[off:off + r * c * _ESZ[dt_]]
                .bitcast(_BDT[dt_]).rearrange("(p f) -> p f", p=r))

    with tile.TileContext(nc) as tc, ExitStack() as ctx:
        const = ctx.enter_context(tc.tile_pool(name="const", bufs=1))
        sb_g = ctx.enter_context(tc.tile_pool(name="sb_g", bufs=2))
        sb_w = ctx.enter_context(tc.tile_pool(name="sb_w", bufs=2))
        sb_dq = ctx.enter_context(tc.tile_pool(name="sb_dq", bufs=3))
        psum = ctx.enter_context(tc.tile_pool(name="psum", bufs=1, space="PSUM"))
        dram = ctx.enter_context(tc.tile_pool(name="dram", bufs=1, space="DRAM"))

        # ---- dequantize the int8 node shard to bf16, then AllGather ----
        gin = dram.tile([sh32, H], bf16, name="gin")
        tab = dram.tile([N, H], bf16, name="tab", addr_space="Shared")
        for t in range(sh32 // P):
            q_t = sb_dq.tile([P, H], i8, tag="dq_q")
            sc_t = sb_dq.tile([P, 1], f32, tag="dq_sc")
            d_t = sb_dq.tile([P, H], bf16, tag="dq_d")
            nc.sync.dma_start(out=q_t[:], in_=bap("shard_q")[t * P:(t + 1) * P, :])
            nc.sync.dma_start(out=sc_t[:], in_=bap("shard_sc")[t * P:(t + 1) * P, :])
            nc.scalar.activation(out=d_t[:], in_=q_t[:],
                                 func=mybir.ActivationFunctionType.Copy,
                                 bias=0.0, scale=sc_t[:])
            nc.sync.dma_start(out=gin[t * P:(t + 1) * P, :], in_=d_t[:])
        nc.gpsimd.collective_compute(
            "AllGather", mybir.AluOpType.bypass,
            replica_groups=[list(range(N_CORES))],
            ins=[gin[:shard, :]], outs=[tab[:]])
        tabs = (tab[:half, :], tab[half:, :])

        # ---- constants / parameters into SBUF ----
        iota_t = const.tile([P, P], u8)
        ident_t = const.tile([P, P], bf16)
        gamma_sb = const.tile([P, H], f32)
        beta_sb = const.tile([P, H], f32)
        wmsg_t = const.tile([H, H], bf16)
        wih_t = const.tile([H, 3 * H], bf16)
        whh_t = const.tile([H, 3 * H], bf16)
        bih_sb = const.tile([H, 3], f32)
        bhh_sb = const.tile([H, 3], f32)
        idx_ts = [const.tile([P, n_tiles_s[s] * 8], i16, name=f"idx_t{s}")
                  for s in (0, 1)]
        dstoff_ts = [const.tile([P, n_tiles_s[s]], u8, name=f"dstoff_t{s}")
                     for s in (0, 1)]
        eps_t = const.tile([P, 1], f32)
        qbias_t = const.tile([P, 1], f32)
        nc.vector.memset(qbias_t[:], Q_BIAS)
        for t, d in ((ident_t, "ident"), (wmsg_t, "wmsgT"), (wih_t, "wihT"),
                     (whh_t, "whhT"), (bih_sb, "bih_t"), (bhh_sb, "bhh_t"),
                     (dstoff_ts[0], "dst_lo"), (dstoff_ts[1], "dst_hi")):
            nc.sync.dma_start(out=t[:], in_=bap(d))
        # single-row sections: load row 0, then log2 partition-doubling copies
        for t, d in ((iota_t, "iota"), (gamma_sb, "gamma_t"), (beta_sb, "beta_t")):
            nc.sync.dma_start(out=t[0:1, :], in_=bap(d))
            k = 1
            while k < P:
                nc.sync.dma_start(out=t[k:2 * k, :], in_=t[0:k, :])
                k *= 2
        # replicate the wrapped idx tables across the 8 gpsimd channels
        for s, nm in ((0, "idx_lo"), (1, "idx_hi")):
            for r in range(8):
                nc.sync.dma_start(out=idx_ts[s][r * 16:(r + 1) * 16, :],
                                  in_=bap(nm))
        nc.vector.memset(eps_t[:], 1e-5)
        if has_bias:
            deg_sb = const.tile([1, shard_pad], bf16)
            bmsg_sb = const.tile([1, H], bf16)
            nc.sync.dma_start(out=deg_sb[:], in_=bap("deg"))
            nc.sync.dma_start(out=bmsg_sb[:], in_=bap("bmsg_row"))

        # ---- phase 1: transposed node shard (resident) + mean partials ----
        nodesT = const.tile([P, shard_pad], bf16)
        if sh32 < shard_pad:
            nc.vector.memset(nodesT[:, sh32:], 0.0)
        nc.sync.dma_start(out=nodesT[:, :sh32], in_=gin[:], transpose=True)

        part13 = const.tile([P, nsb], f32)
        nc.vector.tensor_reduce(
            out=part13[:], in_=nodesT[:].rearrange("p (s d) -> p s d", s=nsb),
            axis=mybir.AxisListType.X, op=mybir.AluOpType.add)
        musum = const.tile([P, 1], f32)
        nc.vector.tensor_reduce(out=musum[:], in_=part13[:],
                                axis=mybir.AxisListType.X, op=mybir.AluOpType.add)

        mu_in = dram.tile([P, 1], f32)
        mu_out = dram.tile([P, 1], f32, addr_space="Shared")
        nc.sync.dma_start(out=mu_in[:], in_=musum[:])
        nc.gpsimd.collective_compute(
            "AllReduce", mybir.AluOpType.add,
            replica_groups=[list(range(N_CORES))],
            ins=[mu_in[:]], outs=[mu_out[:]])
        mu_t = const.tile([P, 1], f32)
        nc.sync.dma_start(out=mu_t[:], in_=mu_out[:])
        mu_bf = const.tile([P, 1], bf16)
        nc.vector.tensor_scalar(out=mu_bf[:], in0=mu_t[:], scalar1=1.0 / N,
                                scalar2=None, op0=mybir.AluOpType.mult)

        # gate biases: biasB[:,g] = W_ih_g @ mu + b_ih_g + b_hh_g (for r,z)
        #              biasA[:,2] = W_ih_n @ mu + b_ih_n  (for n-gate tanh)
        ps_mu = psum.tile([P, 3], f32, tag="ps_r")
        for g in range(3):
            nc.tensor.matmul(out=ps_mu[:, g:g + 1], lhsT=wih_t[:, g * H:(g + 1) * H],
                             rhs=mu_bf[:], start=True, stop=True)
        biasA = const.tile([P, 3], f32)
        biasB = const.tile([P, 3], f32)
        nc.vector.tensor_add(out=biasA[:], in0=ps_mu[:], in1=bih_sb[:])
        nc.vector.tensor_add(out=biasB[:], in0=biasA[:], in1=bhh_sb[:])

        # ---- phase 2: per super-block pipeline ----
        for sb in range(nsb):
            w0 = sb * WPSB
            w_end = min(w0 + WPSB, nw)

            raw_ps = psum.tile([P, SB], f32, tag="ps_raw")
            g_ts, s_ts, t_bases = [None, None], [None, None], [0, 0]
            for s in (0, 1):
                if w0 >= nw:
                    t_bases[s] = n_tiles_s[s]
                    continue
                t_bases[s] = wstart_s[s][w0] // P
                tsb = wstart_s[s][w_end] // P - t_bases[s]
                if tsb == 0:
                    continue
                g_ts[s] = sb_g.tile([P, tsb, P], bf16, tag=f"g{s}",
                                    name=f"g{s}_{sb}")
                nc.gpsimd.dma_gather(
                    out_ap=g_ts[s][:], in_ap=tabs[s],
                    idxs_ap=idx_ts[s][:, t_bases[s] * 8:(t_bases[s] + tsb) * 8],
                    num_idxs=tsb * P, num_idxs_reg=tsb * P, elem_size=H,
                    single_packet=False)
                s_ts[s] = sb_g.tile([P, tsb, P], bf16, tag=f"s{s}",
                                    name=f"s{s}_{sb}")

            for wi in range(WPSB):
                w = w0 + wi
                ntw = (tw[w][0], tw[w][1]) if w < nw else (0, 0)
                nmm = ntw[0] + ntw[1]
                if nmm == 0:
                    nc.vector.memset(raw_ps[:, wi * WIN:(wi + 1) * WIN], 0.0)
                    continue
                j = 0
                for s in (0, 1):
                    if ntw[s] == 0:
                        continue
                    wt0 = wstart_s[s][w] // P - t_bases[s]  # sb-local tile idx
                    # one-hot for this window/stream (DVE, broadcast APs)
                    s_sl = s_ts[s][:, wt0:wt0 + ntw[s], :]
                    dst_sl = dstoff_ts[s][:, t_bases[s] + wt0:
                                          t_bases[s] + wt0 + ntw[s]]
                    dst_b = bass.AP(tensor=dst_sl.tensor, offset=dst_sl.offset,
                                    ap=[dst_sl.ap[0], dst_sl.ap[1], [0, P]])
                    iota_b = bass.AP(tensor=iota_t.tensor, offset=iota_t.offset,
                                     ap=[iota_t.ap[0], [0, ntw[s]], iota_t.ap[1]])
                    nc.vector.tensor_tensor(out=s_sl, in0=iota_b, in1=dst_b,
                                            op=mybir.AluOpType.is_equal)
                    for k in range(ntw[s]):
                        t_loc = wt0 + k
                        nc.tensor.matmul(out=raw_ps[:, wi * WIN:(wi + 1) * WIN],
                                         lhsT=g_ts[s][:, t_loc, :],
                                         rhs=s_ts[s][:, t_loc, :],
                                         start=(j == 0), stop=(j == nmm - 1))
                        j += 1

            # messages^T = W_msg @ raw^T (+ b_msg (x) deg for nonzero b_msg)
            rawT_sb = sb_w.tile([P, SB], bf16, tag="rawT")
            nc.scalar.copy(out=rawT_sb[:], in_=raw_ps[:])
            msg_ps = psum.tile([P, SB], f32, tag="ps_msg")
            nc.tensor.matmul(out=msg_ps[:], lhsT=wmsg_t[:], rhs=rawT_sb[:],
                             start=True, stop=not has_bias)
            if has_bias:
                nc.tensor.matmul(out=msg_ps[:], lhsT=bmsg_sb[:],
                                 rhs=deg_sb[:, sb * SB:(sb + 1) * SB],
                                 start=False, stop=True)
            msgT_sb = sb_w.tile([P, SB], bf16, tag="msgT")
            nc.scalar.copy(out=msgT_sb[:], in_=msg_ps[:])

            # row-major messages for the final residual
            msgrow_ps = psum.tile([P, WPSB, P], bf16, tag="ps_row", bufs=2)
            for j in range(WPSB):
                nc.tensor.transpose(out=msgrow_ps[:, j, :],
                                    in_=msgT_sb[:, j * P:(j + 1) * P],
                                    identity=ident_t[:])

            # GRU gates
            nsl = nodesT[:, sb * SB:(sb + 1) * SB]
            ps_r = psum.tile([P, SB], f32, tag="ps_r")
            ps_z = psum.tile([P, SB], f32, tag="ps_z")
            ps_in = psum.tile([P, SB], f32, tag="ps_in")
            ps_hn = psum.tile([P, SB], f32, tag="ps_hn")
            nc.tensor.matmul(out=ps_r[:], lhsT=wih_t[:, 0:H], rhs=msgT_sb[:],
                             start=True, stop=False)
            nc.tensor.matmul(out=ps_r[:], lhsT=whh_t[:, 0:H], rhs=nsl,
                             start=False, stop=True)
            nc.tensor.matmul(out=ps_z[:], lhsT=wih_t[:, H:2 * H], rhs=msgT_sb[:],
                             start=True, stop=False)
            nc.tensor.matmul(out=ps_z[:], lhsT=whh_t[:, H:2 * H], rhs=nsl,
                             start=False, stop=True)
            nc.tensor.matmul(out=ps_in[:], lhsT=wih_t[:, 2 * H:3 * H],
                             rhs=msgT_sb[:], start=True, stop=True)
            nc.tensor.matmul(out=ps_hn[:], lhsT=whh_t[:, 2 * H:3 * H], rhs=nsl,
                             start=True, stop=True)

            r_sb = sb_w.tile([P, SB], bf16, tag="r")
            z_sb = sb_w.tile([P, SB], bf16, tag="z")
            hnb_sb = sb_w.tile([P, SB], bf16, tag="hnb")
            nc.scalar.activation(out=r_sb[:], in_=ps_r[:],
                                 func=mybir.ActivationFunctionType.Sigmoid,
                                 bias=biasB[:, 0:1], scale=1.0)
            nc.scalar.activation(out=z_sb[:], in_=ps_z[:],
                                 func=mybir.ActivationFunctionType.Sigmoid,
                                 bias=biasB[:, 1:2], scale=1.0)
            nc.scalar.activation(out=hnb_sb[:], in_=ps_hn[:],
                                 func=mybir.ActivationFunctionType.Identity,
                                 bias=bhh_sb[:, 2:3], scale=1.0)

            t_sb = sb_w.tile([P, SB], bf16, tag="t")
            nc.vector.tensor_mul(out=t_sb[:], in0=r_sb[:], in1=hnb_sb[:])
            s2_sb = sb_w.tile([P, SB], f32, tag="s2")
            nc.vector.tensor_add(out=s2_sb[:], in0=ps_in[:], in1=t_sb[:])
            n_sb = sb_w.tile([P, SB], bf16, tag="n")
            nc.scalar.activation(out=n_sb[:], in_=s2_sb[:],
                                 func=mybir.ActivationFunctionType.Tanh,
                                 bias=biasA[:, 2:3], scale=1.0)
            d_sb = sb_w.tile([P, SB], bf16, tag="d")
            nc.vector.tensor_sub(out=d_sb[:], in0=nsl, in1=n_sb[:])
            zd_sb = sb_w.tile([P, SB], bf16, tag="zd")
            nc.vector.tensor_mul(out=zd_sb[:], in0=z_sb[:], in1=d_sb[:])
            h_sb = sb_w.tile([P, SB], bf16, tag="h")
            nc.vector.tensor_add(out=h_sb[:], in0=n_sb[:], in1=zd_sb[:])

            # transpose h to row-major
            hrow_ps = psum.tile([P, WPSB, P], bf16, tag="ps_row", bufs=2)
            for j in range(WPSB):
                nc.tensor.transpose(out=hrow_ps[:, j, :],
                                    in_=h_sb[:, j * P:(j + 1) * P],
                                    identity=ident_t[:])

            # LayerNorm over features (free axis now)
            st = sb_w.tile([P, WPSB, 6], f32, tag="st")
            mv = sb_w.tile([P, WPSB, 2], f32, tag="mv")
            for j in range(WPSB):
                nc.vector.bn_stats(out=st[:, j, :], in_=hrow_ps[:, j, :])
                nc.vector.bn_aggr(out=mv[:, j, :], in_=st[:, j, :])
            sd = sb_w.tile([P, WPSB], f32, tag="sd")
            nc.scalar.activation(out=sd[:], in_=mv[:, :, 1],
                                 func=mybir.ActivationFunctionType.Sqrt,
                                 bias=eps_t[:], scale=1.0)
            rstd = sb_w.tile([P, WPSB], f32, tag="rstd")
            nc.vector.reciprocal(out=rstd[:], in_=sd[:])
            nb = sb_w.tile([P, WPSB], f32, tag="nb")
            nc.vector.scalar_tensor_tensor(out=nb[:], in0=mv[:, :, 0], scalar=-1.0,
                                           in1=rstd[:], op0=mybir.AluOpType.mult,
                                           op1=mybir.AluOpType.mult)
            xn = sb_w.tile([P, WPSB, P], f32, tag="xn")
            for j in range(WPSB):
                nc.scalar.activation(out=xn[:, j, :], in_=hrow_ps[:, j, :],
                                     func=mybir.ActivationFunctionType.Identity,
                                     bias=nb[:, j:j + 1], scale=rstd[:, j:j + 1])

            # out = xn * gamma + beta + messages
            gam_b = bass.AP(tensor=gamma_sb.tensor, offset=gamma_sb.offset,
                            ap=[gamma_sb.ap[0], [0, WPSB], gamma_sb.ap[1]])
            bet_b = bass.AP(tensor=beta_sb.tensor, offset=beta_sb.offset,
                            ap=[beta_sb.ap[0], [0, WPSB], beta_sb.ap[1]])
            bm = sb_w.tile([P, WPSB, P], f32, tag="bm")
            nc.vector.tensor_add(out=bm[:], in0=msgrow_ps[:], in1=bet_b)
            gm = sb_w.tile([P, WPSB, P], f32, tag="gm")
            nc.vector.tensor_mul(out=gm[:], in0=xn[:], in1=gam_b)
            o_sb = sb_w.tile([P, WPSB, P], f32, tag="o")
            nc.vector.tensor_add(out=o_sb[:], in0=gm[:], in1=bm[:])

            # per-row u8 quantization: q = o * (126/rowmax) + Q_BIAS
            ab = sb_w.tile([P, WPSB, P], f32, tag="ab")
            nc.scalar.activation(out=ab[:], in_=o_sb[:],
                                 func=mybir.ActivationFunctionType.Abs,
                                 bias=0.0, scale=1.0)
            mx = sb_w.tile([P, WPSB], f32, tag="mx")
            nc.vector.tensor_reduce(out=mx[:], in_=ab[:],
                                    axis=mybir.AxisListType.X,
                                    op=mybir.AluOpType.max)
            mxg = sb_w.tile([P, WPSB], f32, tag="mxg")
            nc.vector.tensor_scalar(out=mxg[:], in0=mx[:], scalar1=1e-12,
                                    scalar2=None, op0=mybir.AluOpType.max)
            qs = sb_w.tile([P, WPSB], f32, tag="qs")
            nc.vector.reciprocal(out=qs[:], in_=mxg[:])
            qs2 = sb_w.tile([P, WPSB], f32, tag="qs2")
            nc.vector.tensor_scalar(out=qs2[:], in0=qs[:], scalar1=126.0,
                                    scalar2=None, op0=mybir.AluOpType.mult)
            isc = sb_w.tile([P, WPSB], f32, tag="isc")
            nc.vector.tensor_scalar(out=isc[:], in0=mxg[:], scalar1=1.0 / 126.0,
                                    scalar2=None, op0=mybir.AluOpType.mult)
            q_sb = sb_w.tile([P, WPSB, P], u8, tag="q")
            for j in range(WPSB):
                nc.scalar.activation(out=q_sb[:, j, :], in_=o_sb[:, j, :],
                                     func=mybir.ActivationFunctionType.Identity,
                                     bias=qbias_t[:], scale=qs2[:, j:j + 1])

            # store (u8 quants + packed f32 inv-scales, real shard rows only)
            rows0 = sb * SB
            valid = min(SB, shard - rows0)
            jfull = valid // P
            prem = valid % P
            if jfull > 0:
                nc.sync.dma_start(
                    out=out_d[rows0:rows0 + jfull * P, 0:P]
                        .rearrange("(j p) f -> p j f", p=P),
                    in_=q_sb[:, 0:jfull, :])
                nc.sync.dma_start(
                    out=out_d[rows0:rows0 + jfull * P, P:P + 4].bitcast(f32)
                        .rearrange("(j p) f -> p j f", p=P),
                    in_=isc[:, 0:jfull].rearrange("p (j o) -> p j o", o=1))
            if prem > 0:
                nc.sync.dma_start(
                    out=out_d[rows0 + jfull * P:rows0 + valid, 0:P]
                        .rearrange("(j p) f -> p j f", j=1),
                    in_=q_sb[0:prem, jfull:jfull + 1, :])
                nc.sync.dma_start(
                    out=out_d[rows0 + jfull * P:rows0 + valid, P:P + 4]
                        .bitcast(f32).rearrange("(j p) f -> p j f", j=1),
                    in_=isc[0:prem, jfull:jfull + 1]
                        .rearrange("p (j o) -> p j o", o=1))

    nc.finalize()
    return nc


_CACHE = {}


def _get_program(meta):
    key = (meta["N"], meta["H"], meta["n_tiles_lo"], meta["n_tiles_hi"],
           meta["has_bias"], tuple(tuple(x) for x in meta["tw"]))
    if key not in _CACHE:
        _CACHE[key] = _build_program(meta)
    return _CACHE[key]


def kernel(**inputs):
    in_maps, meta = _host_prep(**inputs)
    nc = _get_program(meta)
    res = run_bass_kernel_spmd(nc, in_maps, core_ids=list(range(N_CORES)))
    parts = [_unpack_out(res.results[c]["out_shard"]) for c in range(N_CORES)]
    return np.concatenate(parts, axis=0).astype(np.float32)


# revision 35
# speedup vs baseline: 1.3406x; 1.3406x over previous
"""NodeMPNN (message passing + GRU + LayerNorm) on 8 Trainium2 NeuronCores.

Strategy (dst-sharded graph parallel, transfer-minimized):
  - Nodes/edges sharded by destination node across 8 cores (6250 dst/core).
  - Host ships each core ONLY its own node shard, int8-quantized with a
    per-row f32 scale; the device dequantizes to bf16 (ACT per-partition
    scale) and rebuilds the full node table in every core's HBM with an
    on-device AllGather over NeuronLink (the "halo exchange"), so
    host->device traffic is ~1/16th of replicating bf16 tables.
  - ALL per-core inputs are packed into a single u8 blob parameter (the axon
    tunnel charges ~25ms fixed latency per jit argument; one blob instead of
    ~17 arrays). Sections are sliced on device via bitcast APs.
  - Source-feature gathers are local indirect-DMA reads of the gathered table.
  - Linearity trick: segment_sum(nodes[src] @ W^T) = segment_sum(nodes[src]) @ W^T,
    so we gather raw node rows and apply W_msg once per 512-dst block.
  - Segment sum via PE: edges sorted by dst, padded per 128-dst window;
    one-hot selection matrices built on DVE (iota is_equal against host-provided
    dst offsets); PSUM accumulates G^T @ S = messages^T per window. Pad slots
    gather row 0 with dst offset 255 (matches no one-hot column).
  - int16 gather indices: table split at row 25000 into lo/hi views (two
    streams) so indices fit int16. Index tables are shipped once ([16, n*8])
    and replicated across the 8 gpsimd channels on device.
  - GRU gates computed in transposed (feature-major) layout: gate = W_ih@msg^T +
    W_hh@nodes^T accumulated in PSUM; mean-node term folded into per-feature gate
    biases (partial sums AllReduced across cores).
  - LayerNorm row-major after PE transposes, bn_stats/bn_aggr + ACT apply.
  - Output quantized on device to u8 with a per-row f32 scale (132 B/row vs
    512 B f32): halves-again the result download AND the donated-zeros upload
    that PJRT ships for output buffers. Dequantized to f32 on host; adds
    ~0.6% rms error (total ~7e-3 « 2e-2 gate).
"""

import sys

sys.path.insert(0, "/opt/trn_rl_repo")

from contextlib import ExitStack

import numpy as np
import ml_dtypes

import concourse.bass as bass
import concourse.bacc as bacc
import concourse.tile as tile
from concourse import mybir
from concourse.bass_utils import run_bass_kernel_spmd

BF16 = ml_dtypes.bfloat16
P = 128
N_CORES = 8
WIN = 128          # dst window (one-hot width)
SB = 512           # dst super-block (PSUM free dim)
PAD_OFF = 255.0    # dst offset for pad slots: never matches iota 0..127
OUT_COLS = P + 4   # per-row output: 128 u8 quants + packed f32 inv-scale
Q_BIAS = 128.0     # u8 offset; HW ACT f32->u8 convert rounds to nearest


def _unpack_out(arr):
    """[rows, 132] u8 -> [rows, 128] f32: (q - 128) * row_inv_scale."""
    q = arr[:, :P].astype(np.float32) - 128.0
    sc = np.ascontiguousarray(arr[:, P:P + 4]).view(np.float32)
    return q * sc

_NPDT = {"bf16": BF16, "f32": np.float32, "i16": np.int16, "u8": np.uint8,
         "i8": np.int8}


def _layout(meta):
    """Section layout of the packed input blob: name -> (offset, rows, cols, dtype)."""
    H, shard_pad = meta["H"], meta["shard_pad"]
    sh32 = meta["shard32"]
    ntl, nth = meta["n_tiles_lo"], meta["n_tiles_hi"]
    secs = [
        ("shard_q", sh32, H, "i8"),
        ("shard_sc", sh32, 1, "f32"),
        ("idx_lo", 16, ntl * 8, "i16"),
        ("idx_hi", 16, nth * 8, "i16"),
        ("dst_lo", P, ntl, "u8"),
        ("dst_hi", P, nth, "u8"),
        ("ident", P, P, "bf16"),
        ("wmsgT", H, H, "bf16"),
        ("wihT", H, 3 * H, "bf16"),
        ("whhT", H, 3 * H, "bf16"),
        ("iota", 1, P, "u8"),
        ("gamma_t", 1, H, "f32"),
        ("beta_t", 1, H, "f32"),
        ("bih_t", H, 3, "f32"),
        ("bhh_t", H, 3, "f32"),
    ]
    if meta["has_bias"]:
        secs += [("deg", 1, shard_pad, "bf16"), ("bmsg_row", 1, H, "bf16")]
    out, off = {}, 0
    _ESZ = {"f32": 4, "bf16": 2, "i16": 2, "u8": 1, "i8": 1}
    for name, r, c, dt_ in secs:
        nbytes = r * c * _ESZ[dt_]
        out[name] = (off, r, c, dt_)
        off += -(-nbytes // 256) * 256
    return out, off


def _host_prep(nodes, W_msg, b_msg, w_ih, w_hh, b_ih, b_hh, ln_gamma, ln_beta,
               edge_src, edge_dst):
    """Sort/pad edges, build per-core SPMD input blobs and the tile schedule."""
    N, H = nodes.shape
    assert H == P
    assert N % N_CORES == 0
    shard = N // N_CORES                  # dst nodes per core
    shard_pad = -(-shard // SB) * SB      # padded to super-block multiple
    shard32 = -(-shard // 32) * 32        # upload pad (transpose-DMA xbar tile)
    nsb = shard_pad // SB                 # super-blocks per core
    nw = -(-shard // WIN)                 # real dst windows per core

    half = (N + 1) // 2                   # split tables: int16 gather indices
    assert half < 32768 and N - half < 32768

    has_bias = bool(np.any(np.asarray(b_msg) != 0.0))

    # --- group edges by (core, window, stream) ---
    d_s = np.asarray(edge_dst).astype(np.int64)
    s_s = np.asarray(edge_src).astype(np.int64)
    stream = (s_s >= half).astype(np.int64)
    loc = np.where(stream == 0, s_s, s_s - half)

    core = d_s // shard
    within = d_s - core * shard
    w_of = within // WIN
    off_of = within % WIN

    key = (core * nw + w_of) * 2 + stream
    order = np.argsort(key, kind="stable")
    key, loc, off_of, core = key[order], loc[order], off_of[order], core[order]
    w_s = w_of[order]
    st_s = stream[order]

    counts = np.bincount(key, minlength=N_CORES * nw * 2).reshape(N_CORES, nw, 2)
    tw = (counts.max(axis=0) + P - 1) // P           # [nw, 2] tiles per (window, stream)
    n_tiles_s = [int(tw[:, s].sum()) for s in (0, 1)]
    assert n_tiles_s[0] > 0 and n_tiles_s[1] > 0
    wstart_s = []
    for s in (0, 1):
        ws = np.zeros(nw + 1, np.int64)
        ws[1:] = np.cumsum(tw[:, s] * P)
        wstart_s.append(ws)

    starts_flat = np.zeros(N_CORES * nw * 2 + 1, np.int64)
    starts_flat[1:] = np.cumsum(counts.reshape(-1))
    rank = np.arange(d_s.shape[0], dtype=np.int64) - starts_flat[key]
    slot = np.where(st_s == 0, wstart_s[0][w_s], wstart_s[1][w_s]) + rank

    src_arrs, off_arrs = [], []
    for s in (0, 1):
        total = n_tiles_s[s] * P
        sa = np.zeros((N_CORES, total), np.int16)        # pad: gather row 0
        oa = np.full((N_CORES, total), PAD_OFF, np.float32)
        m = st_s == s
        sa[core[m], slot[m]] = loc[m]
        oa[core[m], slot[m]] = off_of[m]
        src_arrs.append(sa)
        off_arrs.append(oa)

    meta = dict(N=N, H=H, half=half, shard=shard, shard_pad=shard_pad,
                shard32=shard32, nsb=nsb,
                nw=nw, n_tiles_lo=n_tiles_s[0], n_tiles_hi=n_tiles_s[1],
                has_bias=has_bias,
                tw=[[int(tw[w, 0]), int(tw[w, 1])] for w in range(nw)],
                wstart_lo=[int(x) for x in wstart_s[0]],
                wstart_hi=[int(x) for x in wstart_s[1]])
    layout, total_bytes = _layout(meta)
    meta["total_bytes"] = total_bytes

    # --- shared (replicated) sections ---
    nodes_f32 = np.asarray(nodes, np.float32)
    shared = {
        "iota": np.arange(P, dtype=np.uint8).reshape(1, P),
        "ident": np.eye(P, dtype=np.float32).astype(BF16),
        "gamma_t": np.asarray(ln_gamma, np.float32).reshape(1, H),
        "beta_t": np.asarray(ln_beta, np.float32).reshape(1, H),
        "wmsgT": np.asarray(W_msg, np.float32).T.astype(BF16),
        "wihT": np.asarray(w_ih, np.float32).T.astype(BF16),
        "whhT": np.asarray(w_hh, np.float32).T.astype(BF16),
        "bih_t": np.asarray(b_ih, np.float32).reshape(3, H).T.astype(np.float32),
        "bhh_t": np.asarray(b_hh, np.float32).reshape(3, H).T.astype(np.float32),
    }
    if has_bias:
        deg_all = np.bincount(d_s, minlength=N).astype(np.float32)
        shared["bmsg_row"] = np.asarray(b_msg, np.float32).reshape(1, H).astype(BF16)

    in_maps = []
    for c in range(N_CORES):
        blob = np.zeros(total_bytes, np.uint8)

        def put(name, arr):
            off, r, cc, dt_ = layout[name]
            a = np.ascontiguousarray(arr, dtype=_NPDT[dt_])
            assert a.shape == (r, cc), (name, a.shape, (r, cc))
            blob[off:off + a.nbytes] = a.view(np.uint8).reshape(-1)

        rows = nodes_f32[c * shard:(c + 1) * shard]
        sc = np.maximum(np.abs(rows).max(axis=1), 1e-30) / 127.0
        q = np.zeros((shard32, H), np.int8)
        q[:shard] = np.clip(np.rint(rows / sc[:, None]), -127, 127)
        scp = np.zeros((shard32, 1), np.float32)
        scp[:shard, 0] = sc
        put("shard_q", q)
        put("shard_sc", scp)
        for s, nm in ((0, "lo"), (1, "hi")):
            flat = src_arrs[s][c]
            # wrapped int16 layout: index i at [i % 16, i // 16]
            put(f"idx_{nm}", flat.reshape(-1, 16).T)
            put(f"dst_{nm}", off_arrs[s][c].reshape(n_tiles_s[s], P).T)
        for k, v in shared.items():
            put(k, v)
        if has_bias:
            dg = np.zeros((1, shard_pad), np.float32)
            dg[0, :shard] = deg_all[c * shard:(c + 1) * shard]
            put("deg", dg)
        in_maps.append({"blob": blob})

    return in_maps, meta


def _build_program(meta):
    N, H, half = meta["N"], meta["H"], meta["half"]
    shard, shard_pad, nsb, nw = meta["shard"], meta["shard_pad"], meta["nsb"], meta["nw"]
    sh32 = meta["shard32"]
    tw = meta["tw"]
    has_bias = meta["has_bias"]
    n_tiles_s = (meta["n_tiles_lo"], meta["n_tiles_hi"])
    wstart_s = (meta["wstart_lo"], meta["wstart_hi"])
    WPSB = SB // WIN  # windows per super-block (4)
    layout, total_bytes = _layout(meta)

    nc = bacc.Bacc("TRN2", target_bir_lowering=False, debug=False,
                   num_devices=N_CORES)
    f32, bf16, i16 = mybir.dt.float32, mybir.dt.bfloat16, mybir.dt.int16
    u8, i8 = mybir.dt.uint8, mybir.dt.int8
    _BDT = {"bf16": bf16, "f32": f32, "i16": i16, "u8": u8, "i8": i8}

    blob_d = nc.declare_dram_parameter("blob", [total_bytes], u8, isOutput=False)
    out_d = nc.declare_dram_parameter("out_shard", [shard, OUT_COLS], u8, isOutput=True)

    _ESZ = {"f32": 4, "bf16": 2, "i16": 2, "u8": 1, "i8": 1}

    def bap(name, rows=None):
        off, r, c, dt_ = layout[name]
        r = rows if rows is not None else r
        return (blob_d[off:off + r * c * _ESZ[dt_]]
                .bitcast(_BDT[dt_]).rearrange("(p f) -> p f", p=r))

    with tile.TileContext(nc) as tc, ExitStack() as ctx:
        const = ctx.enter_context(tc.tile_pool(name="const", bufs=1))
        sb_g = ctx.enter_context(tc.tile_pool(name="sb_g", bufs=2))
        sb_w = ctx.enter_context(tc.tile_pool(name="sb_w", bufs=2))
        sb_dq = ctx.enter_context(tc.tile_pool(name="sb_dq", bufs=3))
        psum = ctx.enter_context(tc.tile_pool(name="psum", bufs=1, space="PSUM"))
        dram = ctx.enter_context(tc.tile_pool(name="dram", bufs=1, space="DRAM"))

        # ---- dequantize the int8 node shard to bf16, then AllGather ----
        gin = dram.tile([sh32, H], bf16, name="gin")
        tab = dram.tile([N, H], bf16, name="tab", addr_space="Shared")
        for t in range(sh32 // P):
            q_t = sb_dq.tile([P, H], i8, tag="dq_q")
            sc_t = sb_dq.tile([P, 1], f32, tag="dq_sc")
            d_t = sb_dq.tile([P, H], bf16, tag="dq_d")
            nc.sync.dma_start(out=q_t[:], in_=bap("shard_q")[t * P:(t + 1) * P, :])
            nc.sync.dma_start(out=sc_t[:], in_=bap("shard_sc")[t * P:(t + 1) * P, :])
            nc.scalar.activation(out=d_t[:], in_=q_t[:],
                                 func=mybir.ActivationFunctionType.Copy,
                                 bias=0.0, scale=sc_t[:])
            nc.sync.dma_start(out=gin[t * P:(t + 1) * P, :], in_=d_t[:])
        nc.gpsimd.collective_compute(
            "AllGather", mybir.AluOpType.bypass,
            replica_groups=[list(range(N_CORES))],
            ins=[gin[:shard, :]], outs=[tab[:]])
        tabs = (tab[:half, :], tab[half:, :])

        # ---- constants / parameters into SBUF ----
        iota_t = const.tile([P, P], u8)
        ident_t = const.tile([P, P], bf16)
        gamma_sb = const.tile([P, H], f32)
        beta_sb = const.tile([P, H], f32)
        wmsg_t = const.tile([H, H], bf16)
        wih_t = const.tile([H, 3 * H], bf16)
        whh_t = const.tile([H, 3 * H], bf16)
        bih_sb = const.tile([H, 3], f32)
        bhh_sb = const.tile([H, 3], f32)
        idx_ts = [const.tile([P, n_tiles_s[s] * 8], i16, name=f"idx_t{s}")
                  for s in (0, 1)]
        dstoff_ts = [const.tile([P, n_tiles_s[s]], u8, name=f"dstoff_t{s}")
                     for s in (0, 1)]
        eps_t = const.tile([P, 1], f32)
        qbias_t = const.tile([P, 1], f32)
        nc.vector.memset(qbias_t[:], Q_BIAS)
        for t, d in ((ident_t, "ident"), (wmsg_t, "wmsgT"), (wih_t, "wihT"),
                     (whh_t, "whhT"), (bih_sb, "bih_t"), (bhh_sb, "bhh_t"),
                     (dstoff_ts[0], "dst_lo"), (dstoff_ts[1], "dst_hi")):
            nc.sync.dma_start(out=t[:], in_=bap(d))
        # single-row sections: load row 0, then log2 partition-doubling copies
        for t, d in ((iota_t, "iota"), (gamma_sb, "gamma_t"), (beta_sb, "beta_t")):
            nc.sync.dma_start(out=t[0:1, :], in_=bap(d))
            k = 1
            while k < P:
                nc.sync.dma_start(out=t[k:2 * k, :], in_=t[0:k, :])
                k *= 2
        # replicate the wrapped idx tables across the 8 gpsimd channels
        for s, nm in ((0, "idx_lo"), (1, "idx_hi")):
            for r in range(8):
                nc.sync.dma_start(out=idx_ts[s][r * 16:(r + 1) * 16, :],
                                  in_=bap(nm))
        nc.vector.memset(eps_t[:], 1e-5)
        if has_bias:
            deg_sb = const.tile([1, shard_pad], bf16)
            bmsg_sb = const.tile([1, H], bf16)
            nc.sync.dma_start(out=deg_sb[:], in_=bap("deg"))
            nc.sync.dma_start(out=bmsg_sb[:], in_=bap("bmsg_row"))

        # ---- phase 1: transposed node shard (resident) + mean partials ----
        nodesT = const.tile([P, shard_pad], bf16)
        if sh32 < shard_pad:
            nc.vector.memset(nodesT[:, sh32:], 0.0)
        nc.sync.dma_start(out=nodesT[:, :sh32], in_=gin[:], transpose=True)

        part13 = const.tile([P, nsb], f32)
        nc.vector.tensor_reduce(
            out=part13[:], in_=nodesT[:].rearrange("p (s d) -> p s d", s=nsb),
            axis=mybir.AxisListType.X, op=mybir.AluOpType.add)
        musum = const.tile([P, 1], f32)
        nc.vector.tensor_reduce(out=musum[:], in_=part13[:],
                                axis=mybir.AxisListType.X, op=mybir.AluOpType.add)

        mu_in = dram.tile([P, 1], f32)
        mu_out = dram.tile([P, 1], f32, addr_space="Shared")
        nc.sync.dma_start(out=mu_in[:], in_=musum[:])
        nc.gpsimd.collective_compute(
            "AllReduce", mybir.AluOpType.add,
            replica_groups=[list(range(N_CORES))],
            ins=[mu_in[:]], outs=[mu_out[:]])
        mu_t = const.tile([P, 1], f32)
        nc.sync.dma_start(out=mu_t[:], in_=mu_out[:])
        mu_bf = const.tile([P, 1], bf16)
        nc.vector.tensor_scalar(out=mu_bf[:], in0=mu_t[:], scalar1=1.0 / N,
                                scalar2=None, op0=mybir.AluOpType.mult)

        # gate biases: biasB[:,g] = W_ih_g @ mu + b_ih_g + b_hh_g (for r,z)
        #              biasA[:,2] = W_ih_n @ mu + b_ih_n  (for n-gate tanh)
        ps_mu = psum.tile([P, 3], f32, tag="ps_r")
        for g in range(3):
            nc.tensor.matmul(out=ps_mu[:, g:g + 1], lhsT=wih_t[:, g * H:(g + 1) * H],
                             rhs=mu_bf[:], start=True, stop=True)
        biasA = const.tile([P, 3], f32)
        biasB = const.tile([P, 3], f32)
        nc.vector.tensor_add(out=biasA[:], in0=ps_mu[:], in1=bih_sb[:])
        nc.vector.tensor_add(out=biasB[:], in0=biasA[:], in1=bhh_sb[:])

        # ---- phase 2: per super-block pipeline ----
        for sb in range(nsb):
            w0 = sb * WPSB
            w_end = min(w0 + WPSB, nw)

            raw_ps = psum.tile([P, SB], f32, tag="ps_raw")
            g_ts, s_ts, t_bases = [None, None], [None, None], [0, 0]
            for s in (0, 1):
                if w0 >= nw:
                    t_bases[s] = n_tiles_s[s]
                    continue
                t_bases[s] = wstart_s[s][w0] // P
                tsb = wstart_s[s][w_end] // P - t_bases[s]
                if tsb == 0:
                    continue
                g_ts[s] = sb_g.tile([P, tsb, P], bf16, tag=f"g{s}",
                                    name=f"g{s}_{sb}")
                nc.gpsimd.dma_gather(
                    out_ap=g_ts[s][:], in_ap=tabs[s],
                    idxs_ap=idx_ts[s][:, t_bases[s] * 8:(t_bases[s] + tsb) * 8],
                    num_idxs=tsb * P, num_idxs_reg=tsb * P, elem_size=H,
                    single_packet=False)
                s_ts[s] = sb_g.tile([P, tsb, P], bf16, tag=f"s{s}",
                                    name=f"s{s}_{sb}")

            for wi in range(WPSB):
                w = w0 + wi
                ntw = (tw[w][0], tw[w][1]) if w < nw else (0, 0)
                nmm = ntw[0] + ntw[1]
                if nmm == 0:
                    nc.vector.memset(raw_ps[:, wi * WIN:(wi + 1) * WIN], 0.0)
                    continue
                j = 0
                for s in (0, 1):
                    if ntw[s] == 0:
                        continue
                    wt0 = wstart_s[s][w] // P - t_bases[s]  # sb-local tile idx
                    # one-hot for this window/stream (DVE, broadcast APs)
                    s_sl = s_ts[s][:, wt0:wt0 + ntw[s], :]
                    dst_sl = dstoff_ts[s][:, t_bases[s] + wt0:
                                          t_bases[s] + wt0 + ntw[s]]
                    dst_b = bass.AP(tensor=dst_sl.tensor, offset=dst_sl.offset,
                                    ap=[dst_sl.ap[0], dst_sl.ap[1], [0, P]])
                    iota_b = bass.AP(tensor=iota_t.tensor, offset=iota_t.offset,
                                     ap=[iota_t.ap[0], [0, ntw[s]], iota_t.ap[1]])
                    nc.vector.tensor_tensor(out=s_sl, in0=iota_b, in1=dst_b,
                                            op=mybir.AluOpType.is_equal)
                    for k in range(ntw[s]):
                        t_loc = wt0 + k
                        nc.tensor.matmul(out=raw_ps[:, wi * WIN:(wi + 1) * WIN],
                                         lhsT=g_ts[s][:, t_loc, :],
                                         rhs=s_ts[s][:, t_loc, :],
                                         start=(j == 0), stop=(j == nmm - 1))
                        j += 1

            # messages^T = W_msg @ raw^T (+ b_msg (x) deg for nonzero b_msg)
            rawT_sb = sb_w.tile([P, SB], bf16, tag="rawT")
            nc.scalar.copy(out=rawT_sb[:], in_=raw_ps[:])
            msg_ps = psum.tile([P, SB], f32, tag="ps_msg")
            nc.tensor.matmul(out=msg_ps[:], lhsT=wmsg_t[:], rhs=rawT_sb[:],
                             start=True, stop=not has_bias)
            if has_bias:
                nc.tensor.matmul(out=msg_ps[:], lhsT=bmsg_sb[:],
                                 rhs=deg_sb[:, sb * SB:(sb + 1) * SB],
                                 start=False, stop=True)
            msgT_sb = sb_w.tile([P, SB], bf16, tag="msgT")
            nc.scalar.copy(out=msgT_sb[:], in_=msg_ps[:])

            # row-major messages for the final residual
            msgrow_ps = psum.tile([P, WPSB, P], bf16, tag="ps_row", bufs=2)
            for j in range(WPSB):
                nc.tensor.transpose(out=msgrow_ps[:, j, :],
                                    in_=msgT_sb[:, j * P:(j + 1) * P],
                                    identity=ident_t[:])

            # GRU gates
            nsl = nodesT[:, sb * SB:(sb + 1) * SB]
            ps_r = psum.tile([P, SB], f32, tag="ps_r")
            ps_z = psum.tile([P, SB], f32, tag="ps_z")
            ps_in = psum.tile([P, SB], f32, tag="ps_in")
            ps_hn = psum.tile([P, SB], f32, tag="ps_hn")
            nc.tensor.matmul(out=ps_r[:], lhsT=wih_t[:, 0:H], rhs=msgT_sb[:],
                             start=True, stop=False)
            nc.tensor.matmul(out=ps_r[:], lhsT=whh_t[:, 0:H], rhs=nsl,
                             start=False, stop=True)
            nc.tensor.matmul(out=ps_z[:], lhsT=wih_t[:, H:2 * H], rhs=msgT_sb[:],
                             start=True, stop=False)
            nc.tensor.matmul(out=ps_z[:], lhsT=whh_t[:, H:2 * H], rhs=nsl,
                             start=False, stop=True)
            nc.tensor.matmul(out=ps_in[:], lhsT=wih_t[:, 2 * H:3 * H],
                             rhs=msgT_sb[:], start=True, stop=True)
            nc.tensor.matmul(out=ps_hn[:], lhsT=whh_t[:, 2 * H:3 * H], rhs=nsl,
                             start=True, stop=True)

            r_sb = sb_w.tile([P, SB], bf16, tag="r")
            z_sb = sb_w.tile([P, SB], bf16, tag="z")
            hnb_sb = sb_w.tile([P, SB], bf16, tag="hnb")
            nc.scalar.activation(out=r_sb[:], in_=ps_r[:],
                                 func=mybir.ActivationFunctionType.Sigmoid,
                                 bias=biasB[:, 0:1], scale=1.0)
            nc.scalar.activation(out=z_sb[:], in_=ps_z[:],
                                 func=mybir.ActivationFunctionType.Sigmoid,
                                 bias=biasB[:, 1:2], scale=1.0)
            nc.scalar.activation(out=hnb_sb[:], in_=ps_hn[:],
                                 func=mybir.ActivationFunctionType.Identity,
                                 bias=bhh_sb[:, 2:3], scale=1.0)

            t_sb = sb_w.tile([P, SB], bf16, tag="t")
            nc.vector.tensor_mul(out=t_sb[:], in0=r_sb[:], in1=hnb_sb[:])
            s2_sb = sb_w.tile([P, SB], f32, tag="s2")
            nc.vector.tensor_add(out=s2_sb[:], in0=ps_in[:], in1=t_sb[:])
            n_sb = sb_w.tile([P, SB], bf16, tag="n")
            nc.scalar.activation(out=n_sb[:], in_=s2_sb[:],
                                 func=mybir.ActivationFunctionType.Tanh,
                                 bias=biasA[:, 2:3], scale=1.0)
            d_sb = sb_w.tile([P, SB], bf16, tag="d")
            nc.vector.tensor_sub(out=d_sb[:], in0=nsl, in1=n_sb[:])
            zd_sb = sb_w.tile([P, SB], bf16, tag="zd")
            nc.vector.tensor_mul(out=zd_sb[:], in0=z_sb[:], in1=d_sb[:])
            h_sb = sb_w.tile([P, SB], bf16, tag="h")
            nc.vector.tensor_add(out=h_sb[:], in0=n_sb[:], in1=zd_sb[:])

            # transpose h to row-major
            hrow_ps = psum.tile([P, WPSB, P], bf16, tag="ps_row", bufs=2)
            for j in range(WPSB):
                nc.tensor.transpose(out=hrow_ps[:, j, :],
                                    in_=h_sb[:, j * P:(j + 1) * P],
                                    identity=ident_t[:])

            # LayerNorm over features (free axis now)
            st = sb_w.tile([P, WPSB, 6], f32, tag="st")
            mv = sb_w.tile([P, WPSB, 2], f32, tag="mv")
            for j in range(WPSB):
                nc.vector.bn_stats(out=st[:, j, :], in_=hrow_ps[:, j, :])
                nc.vector.bn_aggr(out=mv[:, j, :], in_=st[:, j, :])
            sd = sb_w.tile([P, WPSB], f32, tag="sd")
            nc.scalar.activation(out=sd[:], in_=mv[:, :, 1],
                                 func=mybir.ActivationFunctionType.Sqrt,
                                 bias=eps_t[:], scale=1.0)
            rstd = sb_w.tile([P, WPSB], f32, tag="rstd")
            nc.vector.reciprocal(out=rstd[:], in_=sd[:])
            nb = sb_w.tile([P, WPSB], f32, tag="nb")
            nc.vector.scalar_tensor_tensor(out=nb[:], in0=mv[:, :, 0], scalar=-1.0,
                                           in1=rstd[:], op0=mybir.AluOpType.mult,
                                           op1=mybir.AluOpType.mult)
            xn = sb_w.tile([P, WPSB, P], f32, tag="xn")
            for j in range(WPSB):
                nc.scalar.activation(out=xn[:, j, :], in_=hrow_ps[:, j, :],
                                     func=mybir.ActivationFunctionType.Identity,
                                     bias=nb[:, j:j + 1], scale=rstd[:, j:j + 1])

            # out = xn * gamma + beta + messages
            gam_b = bass.AP(tensor=gamma_sb.tensor, offset=gamma_sb.offset,
                            ap=[gamma_sb.ap[0], [0, WPSB], gamma_sb.ap[1]])
            bet_b = bass.AP(tensor=beta_sb.tensor, offset=beta_sb.offset,
                            ap=[beta_sb.ap[0], [0, WPSB], beta_sb.ap[1]])
            bm = sb_w.tile([P, WPSB, P], f32, tag="bm")
            nc.vector.tensor_add(out=bm[:], in0=msgrow_ps[:], in1=bet_b)
            gm = sb_w.tile([P, WPSB, P], f32, tag="gm")
            nc.vector.tensor_mul(out=gm[:], in0=xn[:], in1=gam_b)
            o_sb = sb_w.tile([P, WPSB, P], f32, tag="o")
            nc.vector.tensor_add(out=o_sb[:], in0=gm[:], in1=bm[:])

            # per-row u8 quantization: q = o * (126/rowmax) + Q_BIAS
            ab = sb_w.tile([P, WPSB, P], f32, tag="ab")
            nc.scalar.activation(out=ab[:], in_=o_sb[:],
                                 func=mybir.ActivationFunctionType.Abs,
                                 bias=0.0, scale=1.0)
            mx = sb_w.tile([P, WPSB], f32, tag="mx")
            nc.vector.tensor_reduce(out=mx[:], in_=ab[:],
                                    axis=mybir.AxisListType.X,
                                    op=mybir.AluOpType.max)
            mxg = sb_w.tile([P, WPSB], f32, tag="mxg")
            nc.vector.tensor_scalar(out=mxg[:], in0=mx[:], scalar1=1e-12,
                                    scalar2=None, op0=mybir.AluOpType.max)
            qs = sb_w.tile([P, WPSB], f32, tag="qs")
            nc.vector.reciprocal(out=qs[:], in_=mxg[:])
            qs2 = sb_w.tile([P, WPSB], f32, tag="qs2")
            nc.vector.tensor_scalar(out=qs2[:], in0=qs[:], scalar1=126.0,
                                    scalar2=None, op0=mybir.AluOpType.mult)
            isc = sb_w.tile([P, WPSB], f32, tag="isc")
            nc.vector.tensor_scalar(out=isc[:], in0=mxg[:], scalar1=1.0 / 126.0,
                                    scalar2=None, op0=mybir.AluOpType.mult)
            q_sb = sb_w.tile([P, WPSB, P], u8, tag="q")
            for j in range(WPSB):
                nc.scalar.activation(out=q_sb[:, j, :], in_=o_sb[:, j, :],
                                     func=mybir.ActivationFunctionType.Identity,
                                     bias=qbias_t[:], scale=qs2[:, j:j + 1])

            # store (u8 quants + packed f32 inv-scales, real shard rows only)
            rows0 = sb * SB
            valid = min(SB, shard - rows0)
            jfull = valid // P
            prem = valid % P
            if jfull > 0:
                nc.sync.dma_start(
                    out=out_d[rows0:rows0 + jfull * P, 0:P]
                        .rearrange("(j p) f -> p j f", p=P),
                    in_=q_sb[:, 0:jfull, :])
                nc.sync.dma_start(
                    out=out_d[rows0:rows0 + jfull * P, P:P + 4].bitcast(f32)
                        .rearrange("(j p) f -> p j f", p=P),
                    in_=isc[:, 0:jfull].rearrange("p (j o) -> p j o", o=1))
            if prem > 0:
                nc.sync.dma_start(
                    out=out_d[rows0 + jfull * P:rows0 + valid, 0:P]
                        .rearrange("(j p) f -> p j f", j=1),
                    in_=q_sb[0:prem, jfull:jfull + 1, :])
                nc.sync.dma_start(
                    out=out_d[rows0 + jfull * P:rows0 + valid, P:P + 4]
                        .bitcast(f32).rearrange("(j p) f -> p j f", j=1),
                    in_=isc[0:prem, jfull:jfull + 1]
                        .rearrange("p (j o) -> p j o", o=1))

    nc.finalize()
    return nc


_CACHE = {}


def _get_program(meta):
    key = (meta["N"], meta["H"], meta["n_tiles_lo"], meta["n_tiles_hi"],
           meta["has_bias"], tuple(tuple(x) for x in meta["tw"]))
    if key not in _CACHE:
        _CACHE[key] = _build_program(meta)
    return _CACHE[key]


def kernel(**inputs):
    in_maps, meta = _host_prep(**inputs)
    nc = _get_program(meta)
    res = run_bass_kernel_spmd(nc, in_maps, core_ids=list(range(N_CORES)))
    parts = [_unpack_out(res.results[c]["out_shard"]) for c in range(N_CORES)]
    return np.concatenate(parts, axis=0).astype(np.float32)
